# revision 30
# baseline (speedup 1.0000x reference)
"""Trainium2 Bass kernel for CifNet conv-QKV self-attention.

Sharding: 8 cores = 4 (batch) x 2 (head-groups of 4 heads).
Each core computes, for its batch sample b and head-group g:
  - q/k/v = conv3x3(x, w{q,k,v}[g*256:(g+1)*256])   (256 out-channels = 4 heads)
  - per-head attention over hw=2304 positions (softmax without max-subtraction,
    denominator fused into the AV matmul via an appended ones-column on V^T)
  - partial o-conv: conv3x3(attn_out, wo[:, g*256:(g+1)*256])  -> [256, 2304] fp32
Host sums the two head-group partials per batch sample.

Convs are expressed as 9 shifted matmuls (one per tap) accumulating in PSUM,
with the input pre-padded to [C, 50, 50] on the host. All matmuls run in bf16
with fp32 PSUM accumulation.

Every matmul keeps a 128-row stationary config: Q is stored per-head in
zero-padded [128, hw] tiles (the other head's 64 partitions zero) so score
matmuls contract over the natural two-head K stationary without flipping the
PE between 64-row and 128-row tile configs (the reconfig costs ~90ns each).
Input DMAs are ordered x -> wv -> wq -> wk -> wo so the first conv can start
as early as possible, and the PE is warmed up on dummy matmuls during the DMA
wait so the p-state ramp completes before real work arrives. The o-conv's second
input-half pass is dripped into the tail of attention m1 instead of running
as a serial epilogue.
"""

from contextlib import ExitStack

import numpy as np
import ml_dtypes

# problem shape (hardcoded per contract)
B, C, H, W = 4, 256, 48, 48
HW = H * W              # 2304
NCORES = 8
RT = 8                  # output rows per spatial tile
NT = RT * W             # 384 columns per matmul
NROW = H // RT          # 6 spatial tiles
NKJ = HW // 128         # 18 key tiles

_cached = None


def _build():
    """Build and compile the per-core SPMD Bass program (cached)."""
    global _cached
    if _cached is not None:
        return _cached

    import concourse.bass as bass  # noqa: F401
    import concourse.tile as tile
    from concourse import bacc, mybir
    from concourse.masks import make_identity

    BF = mybir.dt.bfloat16
    F32 = mybir.dt.float32
    EXP = mybir.ActivationFunctionType.Exp

    nc = bacc.Bacc("TRN2", target_bir_lowering=False, debug=False)
    x_d = nc.dram_tensor("xpad", [2, 128, 50, 50], BF, kind="ExternalInput").ap()
    wqkv_d = nc.dram_tensor("wqkv", [3, 9, 2, 128, 256], BF, kind="ExternalInput").ap()
    wo_d = nc.dram_tensor("wo", [9, 2, 128, 256], BF, kind="ExternalInput").ap()
    out_d = nc.dram_tensor("out", [2, 128, HW], F32, kind="ExternalOutput").ap()

    with tile.TileContext(nc) as tc, ExitStack() as ctx:
        konst = ctx.enter_context(tc.tile_pool(name="konst", bufs=1))
        # full 128x128 identity for PE transposes (keeps transposes in the
        # same 128-row PE config as every other matmul)
        ident = konst.tile([128, 128], BF, name="ident")
        make_identity(nc, ident[:])

        x_sb = konst.tile([128, 2, 50, 50], BF, name="x_sb")
        wq_sb = konst.tile([128, 9, 2, 256], BF, name="wq_sb")
        wk_sb = konst.tile([128, 9, 2, 256], BF, name="wk_sb")
        wv_sb = konst.tile([128, 9, 2, 256], BF, name="wv_sb")
        wo_sb = konst.tile([128, 9, 2, 256], BF, name="wo_sb")
        # Q per (m, hh): head hh's channels stay at their natural partitions
        # (64*hh .. 64*hh+64) and the OTHER 64 partitions are zero, so the
        # score matmul can contract over all 128 partitions of the natural
        # two-head K stationary while only head hh contributes. This keeps
        # every matmul in the 128-row PE config (no 64<->128 reconfigs) and
        # keeps all PSUM->SBUF copies partition-aligned.
        qp = [[konst.tile([128, HW], BF, name=f"qp{m}{hh}") for hh in range(2)]
              for m in range(2)]
        k_sb = [konst.tile([128, HW], BF, name=f"k_sb{m}") for m in range(2)]
        v_sb = [konst.tile([128, HW], BF, name=f"v_sb{m}") for m in range(2)]
        # V^T per head: [kj within tile, kj tile, 65]; col 64 holds ones so the
        # AV matmul also produces the softmax denominator in psum row 64.
        vt_sb = [konst.tile([128, NKJ, 65], BF, name=f"vt_sb{h}") for h in range(4)]
        opad = [konst.tile([128, 50, 50], BF, name=f"opad{g}") for g in range(2)]

        # input DMAs, ordered by first use: x kg0 + wv (phase A starts with
        # the v conv over kg0), x kg1 (first read ~9us in), then wq, wk, wo.
        nc.sync.dma_start(x_sb[:, 0], x_d[0])
        for t in range(9):
            nc.sync.dma_start(wv_sb[:, t], wqkv_d[2, t].rearrange("g p o -> p g o"))
        nc.sync.dma_start(x_sb[:, 1], x_d[1])
        for a, w_sb in ((0, wq_sb), (1, wk_sb)):
            for t in range(9):
                nc.sync.dma_start(w_sb[:, t], wqkv_d[a, t].rearrange("g p o -> p g o"))
        for t in range(9):
            nc.sync.dma_start(wo_sb[:, t], wo_d[t].rearrange("g p o -> p g o"))

        for h in range(4):
            nc.gpsimd.memset(vt_sb[h][:], 1.0)
        for g in range(2):
            nc.gpsimd.memset(opad[g][:], 0.0)
        # zero pads for Q (the non-hh head's 64 partitions; first read is the
        # first score matmul, long after these complete)
        for m in range(2):
            for hh in range(2):
                z = 64 * (1 - hh)
                nc.gpsimd.memset(qp[m][hh][z:z + 64, :], 0.0)

        # warm the ACT exp table during the DMA phase (one-time ~2.7us load)
        wrm = konst.tile([1, 8], F32, name="wrm")
        nc.gpsimd.memset(wrm[:], 0.0)
        nc.scalar.activation(wrm[:], wrm[:], EXP, scale=0.125)

        def store_plain(dst):
            def f(r, ps):
                nc.vector.tensor_copy(dst[:, r * NT:(r + 1) * NT], ps[:])
            return f

        def store_split(dsts):
            # partition-aligned: head hh's rows go to the same partitions
            def f(r, ps):
                for hh in range(2):
                    nc.vector.tensor_copy(
                        dsts[hh][64 * hh:64 * hh + 64, r * NT:(r + 1) * NT],
                        ps[64 * hh:64 * hh + 64, :],
                    )
            return f

        # ---------------- phase A: m0 convs + v-m0 transposes ----------------
        def conv_block(m, w_sb, store, cpool, x_src):
            """One full conv output tile-row group: 18 accumulating MMs x 6 rowtiles."""
            ps = [cpool.tile([128, NT], F32, tag="cps", name="cps") for _ in range(NROW)]
            first = True
            for kg in range(2):
                for t in range(9):
                    ky, kx = t // 3, t % 3
                    lhsT = w_sb[:, t, kg, m * 128:(m + 1) * 128]
                    last = (kg == 1 and t == 8)
                    for r in range(NROW):
                        rhs = x_src[:, kg, r * RT + ky: r * RT + ky + RT, kx: kx + W]
                        nc.tensor.matmul(ps[r][:], lhsT, rhs, start=first, stop=last)
                    first = False
            for r in range(NROW):
                store(r, ps[r])

        def transpose_unit(m, kt, tpool, ttag="tps"):
            """Transpose one [128,128] V block (both heads at once)."""
            pt = tpool.tile([128, 128], BF, tag=ttag, name="tps")
            nc.tensor.transpose(
                pt[:], v_sb[m][:, kt * 128:(kt + 1) * 128], ident[:]
            )
            for hh in range(2):
                h = 2 * m + hh
                nc.vector.tensor_copy(
                    vt_sb[h][:, kt, 0:64], pt[:, 64 * hh:64 * hh + 64]
                )

        with tc.tile_pool(name="cpsum", bufs=6, space="PSUM") as cpsum, \
             tc.tile_pool(name="tpsum", bufs=2, space="PSUM") as tpsum:
            # PE warmup: dummy transposes during the input-DMA wait keep the
            # tensor engine continuously busy so the p-state ramp (~3us to max
            # clock) completes before the first conv matmul.
            wps = [tpsum.tile([128, 128], BF, tag="tps", name="wps") for _ in range(2)]
            for i in range(48):
                nc.tensor.transpose(wps[i % 2][:], ident[:], ident[:])

            conv_block(0, wv_sb, store_plain(v_sb[0]), cpsum, x_sb)
            for kt in range(NKJ):
                transpose_unit(0, kt, tpsum)
            conv_block(0, wq_sb, store_split(qp[0]), cpsum, x_sb)
            conv_block(0, wk_sb, store_plain(k_sb[0]), cpsum, x_sb)

        # ---------------- phases B/C/D: attention interleaved with m1 convs
        # and the o-conv, so the PE always has independent work and never
        # blips waiting on the ACT exp (which would throttle its clock).
        osum = [konst.tile([128, HW], F32, name=f"osum{mo}") for mo in range(2)]

        with tc.tile_pool(name="spsum", bufs=2, space="PSUM") as spsum, \
             tc.tile_pool(name="apsum", bufs=2, space="PSUM") as apsum, \
             tc.tile_pool(name="fpsum", bufs=2, space="PSUM") as fpsum, \
             tc.tile_pool(name="esb", bufs=4) as esb, \
             tc.tile_pool(name="osb", bufs=3) as osb, \
             tc.tile_pool(name="nsb", bufs=2) as nsb:

            def conv_row_unit(m, w_sb, store, r):
                """One rowtile of a conv: 18 accumulating MMs into 1 psum bank."""
                ps = fpsum.tile([128, NT], F32, tag="fps", name="fps")
                first = True
                for kg in range(2):
                    for t in range(9):
                        ky, kx = t // 3, t % 3
                        lhsT = w_sb[:, t, kg, m * 128:(m + 1) * 128]
                        rhs = x_sb[:, kg, r * RT + ky: r * RT + ky + RT, kx: kx + W]
                        nc.tensor.matmul(ps[:], lhsT, rhs, start=first,
                                         stop=(kg == 1 and t == 8))
                        first = False
                store(r, ps)

            def oconv_row_unit(mo, r, kg):
                """One rowtile of the o-conv for one input kgroup (9 taps)."""
                ps = fpsum.tile([128, NT], F32, tag="fps", name="fps")
                for t in range(9):
                    ky, kx = t // 3, t % 3
                    lhsT = wo_sb[:, t, kg, mo * 128:(mo + 1) * 128]
                    rhs = opad[kg][:, r * RT + ky: r * RT + ky + RT, kx: kx + W]
                    nc.tensor.matmul(ps[:], lhsT, rhs, start=(t == 0), stop=(t == 8))
                if kg == 0:
                    nc.vector.tensor_copy(osum[mo][:, r * NT:(r + 1) * NT], ps[:])
                else:
                    ot = osb.tile([128, NT], F32, tag="osb", name="osb")
                    nc.vector.tensor_tensor(
                        ot[:], ps[:], osum[mo][:, r * NT:(r + 1) * NT],
                        mybir.AluOpType.add,
                    )
                    nc.sync.dma_start(out_d[mo, :, r * NT:(r + 1) * NT], ot[:])

            def att_unit(m, qi, grp2):
                """Both heads / 2 kj tiles: 4 score MMs (128-row config; the
                natural two-head K stationary is shared by both heads, the
                per-head zero-padded Q selects one head), 2 exps, 4 AV MMs."""
                qsl = slice(qi * NT, (qi + 1) * NT)
                sp = [spsum.tile([128, 2, 512], F32, tag="sps", name="sps")
                      for _ in range(2)]
                for j in range(2):
                    kjt = grp2 * 2 + j
                    for hh in range(2):
                        nc.tensor.matmul(
                            sp[hh][:, j, 0:NT],
                            k_sb[m][:, kjt * 128:(kjt + 1) * 128],
                            qp[m][hh][:, qsl],
                            start=True, stop=True,
                        )
                ets = []
                for hh in range(2):
                    et = esb.tile([128, 2, NT], BF, tag="et", name="et")
                    nc.scalar.activation(et[:], sp[hh][:, :, 0:NT], EXP, scale=0.125)
                    ets.append(et)
                for hh in range(2):
                    h = 2 * m + hh
                    for j in range(2):
                        kjt = grp2 * 2 + j
                        nc.tensor.matmul(
                            av_cur[hh][0:65, :], vt_sb[h][:, kjt, 0:65],
                            ets[hh][:, j, :],
                            start=(kjt == 0), stop=(kjt == NKJ - 1),
                        )

            def normalize(m, qi, hh):
                avf = nsb.tile([128, NT], F32, tag="avf", name="avf")
                nc.vector.tensor_copy(avf[0:65, :], av_cur[hh][0:65, :])
                dn = nsb.tile([1, NT], F32, tag="dn", name="dn")
                nc.sync.dma_start(dn[:], avf[64:65, :])
                rc = nsb.tile([1, NT], F32, tag="rc", name="rc")
                nc.vector.reciprocal_approx_fast(rc[:], dn[:])
                rb = nsb.tile([64, NT], F32, tag="rb", name="rb")
                nc.gpsimd.partition_broadcast(rb[:], rc[:])
                tmp = nsb.tile([64, NT], BF, tag="tmp", name="tmp")
                nc.vector.tensor_mul(tmp[:], avf[0:64, :], rb[:])
                dst = opad[m][64 * hh:64 * hh + 64, qi * RT + 1: qi * RT + RT + 1, 1:49]
                nc.sync.dma_start(dst, tmp[:].rearrange("p (r c) -> p r c", c=W))

            # filler list: PE-only work dripped into the attention stream, in
            # dependency order (v conv first, then its transposes, then q/k)
            fillers_b = (
                [lambda r=r: conv_row_unit(1, wv_sb, store_plain(v_sb[1]), r)
                 for r in range(NROW)]
                + [lambda kt=kt: transpose_unit(1, kt, fpsum, ttag="fps")
                   for kt in range(NKJ)]
                + [lambda r=r: conv_row_unit(1, wq_sb, store_split(qp[1]), r)
                   for r in range(NROW)]
                + [lambda r=r: conv_row_unit(1, wk_sb, store_plain(k_sb[1]), r)
                   for r in range(NROW)]
            )

            def run_attention(m, fillers, post_qi=None):
                """Emit all attention units for head-pair m, interspersing
                rate-paced fillers (dependency-free PE work) plus optional
                qi-anchored fillers (emitted only after a given qi's
                normalizes, so a data-dependent filler can never head-of-line
                block the in-order PE queue on a not-yet-emitted producer)."""
                fi = 0
                n_units = NROW * 9
                ui = 0
                for qi in range(NROW):
                    av_cur[0] = apsum.tile([128, NT], F32, tag="avps", name="avps")
                    av_cur[1] = apsum.tile([128, NT], F32, tag="avps", name="avps")
                    for grp2 in range(9):
                        att_unit(m, qi, grp2)
                        ui += 1
                        # drip PE-only work at a steady rate
                        while fi < len(fillers) and ui * len(fillers) >= (fi + 1) * n_units:
                            fillers[fi]()
                            fi += 1
                        if at_unit and ui in at_unit:
                            for f in at_unit[ui]:
                                f()
                    for hh in range(2):
                        normalize(m, qi, hh)
                    if post_qi and qi in post_qi:
                        for f in post_qi[qi]:
                            f()
                while fi < len(fillers):
                    fillers[fi]()
                    fi += 1

            av_cur = [None, None]
            run_attention(0, fillers_b)

            # phase C: attention m1. Rate-paced fillers: o-conv kg0 rows 0-3
            # (opad[0] is complete, no data deps). qi-anchored fillers: kg1
            # row r reads opad[1] rows 8r..8r+10 = normalizes through qi=r+1,
            # so it's emitted after qi=r+2 (one qi of slack). kg0 rows 4-5
            # are held back to after the last qi so the PE has independent
            # work while the final normalize chain drains.
            # kg0 rows 4-5 ride at the end of the rate-paced list so they land
            # inside the last qi's unit stream, covering the final exp waits
            # and the qi=5 normalize chain with dependency-free PE work.
            fillers_c = [lambda mo=mo, r=r: oconv_row_unit(mo, r, 0)
                         for r in range(NROW) for mo in range(2)]
            # all of kg1 rows 0-3 (they need only qi<=4) run as the post-qi5
            # reserve: ~12us of stall-free PE work bridging the last exps and
            # the final normalize chain before the epilogue's qi5-dependent
            # rows.
            post_qi = {
                5: [lambda mo=mo, r=r: oconv_row_unit(mo, r, 1)
                    for r in range(4) for mo in range(2)],
            }
            run_attention(1, fillers_c, post_qi)

            # phase D: last o-conv kg1 rows + combine + store
            for r in (4, 5):
                for mo in range(2):
                    oconv_row_unit(mo, r, 1)

    nc.compile()
    _cached = nc
    return nc


def make_in_maps(hidden_states, wq, wk, wv, wo):
    """Shard + pre-transform full inputs into 8 per-core input dicts."""
    bf = ml_dtypes.bfloat16
    hidden_states = np.asarray(hidden_states, np.float32)
    in_maps = []
    for core in range(NCORES):
        b, g = core // 2, core % 2
        xp = np.zeros((C, 50, 50), np.float32)
        xp[:, 1:49, 1:49] = hidden_states[b]
        xpad = np.ascontiguousarray(xp.reshape(2, 128, 50, 50)).astype(bf)
        wstk = np.stack(
            [
                np.asarray(w, np.float32)[g * 256:(g + 1) * 256]
                .transpose(2, 3, 1, 0)
                .reshape(9, 2, 128, 256)
                for w in (wq, wk, wv)
            ]
        ).astype(bf)
        wog = (
            np.asarray(wo, np.float32)[:, g * 256:(g + 1) * 256]
            .transpose(2, 3, 1, 0)
            .reshape(9, 2, 128, 256)
            .astype(bf)
        )
        in_maps.append({"xpad": xpad, "wqkv": wstk, "wo": wog})
    return in_maps


def combine_outputs(per_core_outs):
    """Sum the two head-group partials per batch sample."""
    out = np.empty((B, C, H, W), np.float32)
    for b in range(B):
        acc = per_core_outs[2 * b].reshape(C, HW).astype(np.float32) + \
              per_core_outs[2 * b + 1].reshape(C, HW).astype(np.float32)
        out[b] = acc.reshape(C, H, W)
    return out


def kernel(hidden_states, wq, wk, wv, wo):
    from concourse.bass_utils import run_bass_kernel_spmd

    nc = _build()
    in_maps = make_in_maps(hidden_states, wq, wk, wv, wo)
    res = run_bass_kernel_spmd(nc, in_maps, core_ids=list(range(NCORES)))
    return combine_outputs([r["out"] for r in res.results])
